# revision 6
# baseline (speedup 1.0000x reference)
"""DiffVolumeV2 Trainium2 kernel, v8.

out[b,c,d,h,x] = left[b,c,h,x] - right[b,c,h, clip(4x - d + 1, 0, Wr-1)]
with B=4, C=32, H=80, Wl=160, Wr=640, D=48.  10240 independent (b,c,h)
rows sharded contiguously across 8 cores; 10 tiles of 128 partitions.

Same phase-plane decomposition as v7 (d = 4q+s; plane[r][u] = right[4u+r];
front pad = right[0] realizes the clip), but with the planes stored
slot-permuted (slot s holds phase r_s = s^1) so the s-dependence of every
access is affine over (s2, s1), s = 2*s2 + s1:

    out[(4q+2*s2+s1)*WL + x] = left[x] - plane_slot[2*s2+s1][13 - s2 - q + x]

which lets ONE 3-free-dim TensorTensor instruction per s2 cover 2 phases x
12 q x 160 x (the DVE TT encoding S3S3D3 allows 3 free dims; v7's
scalar_tensor_tensor is S2S2D2 and needed 4 instructions + a dummy scalar).
2 subtract instructions per tile instead of 4; the DVE stream is ~98% dense.

v8 structural changes vs v7:
  * All inputs on the two HWDGE rings (sync + scalar queues): tile 0's
    right in quarters split across both rings + left, then tiles 1-9
    alternating rings.  First subtract starts ~2.5us earlier.
  * All output DMAs issued from the GpSimd SWDGE queue (desc-gen on the
    otherwise idle GpSimd engine), so the Act queue runs nothing but
    deinterleaves/pads and the rings carry inputs only.
  * Tail taper [4,3,2,2,1] q-chunks on the last tile, each chunk's output
    DMA'd on completion.
"""

import numpy as np
from concourse import bacc, bass, tile
from concourse.bass_utils import run_bass_kernel_spmd
from concourse.tile_rust import add_dep_helper
import concourse.mybir as mybir

B, C, H, WL, WR, D = 4, 32, 80, 160, 640, 48
N_CORES = 8
R = B * C * H            # 10240 rows
RPC = R // N_CORES       # 1280 rows per core
P = 128
TILES = RPC // P         # 10
PPAD = 13
PW = PPAD + WL           # 173
PLW = 4 * PW
TAPER = [(0, 4), (4, 3), (7, 2), (9, 1), (10, 1), (11, 1)]

_cached = None


def _build() -> bass.Bass:
    nc = bacc.Bacc()
    left_p = nc.declare_dram_parameter("left", [RPC, WL], mybir.dt.float32, isOutput=False)
    right_p = nc.declare_dram_parameter("right", [RPC, WR], mybir.dt.float32, isOutput=False)
    out_p = nc.declare_dram_parameter("out", [RPC, D, WL], mybir.dt.bfloat16, isOutput=True)
    out_flat = out_p[:].rearrange("r d x -> r (d x)")

    def ap(t, off, dims):
        return bass.AP(t.tensor, t.offset + off, [list(t.ap[0])] + dims)

    chains = {}

    def order(key, inst):
        prev = chains.get(key)
        if prev is not None:
            add_dep_helper(inst.ins, prev.ins, sync=False,
                           reason=f"{key} program order")
        chains[key] = inst
        return inst

    def tensor_tensor(out, in0, in1):
        eng = nc.vector
        return eng.add_instruction(
            mybir.InstTensorTensor(
                name=eng.bass.get_next_instruction_name(),
                op=mybir.AluOpType.subtract,
                ins=[eng.lower_ap(in0), eng.lower_ap(in1)],
                outs=[eng.lower_ap(out)],
            )
        )

    with tile.TileContext(nc) as tc:
        with tc.tile_pool(name="inp", bufs=1) as inp_pool, \
             tc.tile_pool(name="ot", bufs=4) as ot_pool:
            rt_all = inp_pool.tile([P, TILES * WR], mybir.dt.float32)
            lt_all = inp_pool.tile([P, TILES * WL], mybir.dt.float32)
            planes = inp_pool.tile([P, TILES * PLW], mybir.dt.float32)

            ringA, ringB = nc.sync, nc.scalar
            HQ = WR // 4     # 160-element quarter of a right row

            def load_right(eng, t, off, n):
                return eng.dma_start(
                    out=ap(rt_all, t * WR + off, [[1, n]]),
                    in_=bass.AP(right_p[:].tensor, t * P * WR + off,
                                [[WR, P], [1, n]]))

            def load_left(eng, t):
                eng.dma_start(
                    out=ap(lt_all, t * WL, [[1, WL]]),
                    in_=bass.AP(left_p[:].tensor, t * P * WL, [[WL, P], [1, WL]]))

            # Tile 0: right quarters 1,2 + left on ring A (deint of the
            # first quarter gates the first subtract), right half 2 on B.
            load_right(ringA, 0, 0, HQ)
            load_left(ringA, 0)
            load_right(ringA, 0, HQ, HQ)
            rh2 = load_right(ringB, 0, 2 * HQ, 2 * HQ)
            load_right(ringA, 1, 0, WR)
            load_left(ringA, 1)
            load_right(ringB, 2, 0, WR)
            load_left(ringB, 2)

            # Tiles 1-9 via the GpSimd SWDGE queue (desc-gen off the
            # ring-issuing queues, which carry the output stream).
            def load_group(t0, nt):
                first = nc.gpsimd.dma_start(
                    out=ap(rt_all, t0 * WR, [[WR, nt], [1, WR]]),
                    in_=bass.AP(right_p[:].tensor, t0 * P * WR,
                                [[WR, P], [WR * P, nt], [1, WR]]))
                nc.gpsimd.dma_start(
                    out=ap(lt_all, t0 * WL, [[WL, nt], [1, WL]]),
                    in_=bass.AP(left_p[:].tensor, t0 * P * WL,
                                [[WL, P], [WL * P, nt], [1, WL]]))
                return first

            g1 = load_group(3, 3)
            load_group(6, 4)
            add_dep_helper(g1.ins, rh2.ins, sync=True,
                           reason="keep head DMA engines clear of bulk input")

            Copy = mybir.ActivationFunctionType.Copy

            def deint(t, xoff, nx):
                # plane_slot[2*s2+s1][PPAD+xoff+u] = right[4*(xoff+u) + 2*s2+1-s1]
                for s2 in range(2):
                    order("act", nc.scalar.activation(
                        ap(planes, t * PLW + 2 * s2 * PW + PPAD + xoff,
                           [[PW, 2], [1, nx]]),
                        ap(rt_all, t * WR + 4 * xoff + 2 * s2 + 1,
                           [[-1, 2], [4, nx]]),
                        Copy))

            def pad(t):
                order("act", nc.scalar.activation(
                    ap(planes, t * PLW, [[PW, 4], [1, PPAD]]),
                    ap(rt_all, t * WR, [[0, 4], [0, PPAD]]),
                    Copy))

            def sub(ot, po, lt_off, s2, q0, nq, xoff=0, nx=WL):
                return order("dve", tensor_tensor(
                    ap(ot, (4 * q0 + 2 * s2) * WL + xoff,
                       [[WL, 2], [4 * WL, nq], [1, nx]]),
                    ap(lt_all, lt_off + xoff, [[0, 2], [0, nq], [1, nx]]),
                    ap(planes, po + s2 * (2 * PW - 1) + PPAD - q0 + xoff,
                       [[PW, 2], [-1, nq], [1, nx]])))

            ring_i = [0]

            def out_dma(r0, ot, c0, cw):
                eng = ringA if ring_i[0] % 2 == 0 else ringB
                ring_i[0] += 1
                eng.dma_start(out=out_flat[r0:r0 + P, c0:c0 + cw],
                              in_=ot[:, c0:c0 + cw])

            for t in range(TILES):
                r0 = t * P
                po = t * PLW
                if t == 0:
                    pad(0)
                    deint(0, 0, WL // 4)
                    deint(0, WL // 4, WL // 4)
                    deint(0, WL // 2, WL // 2)
                else:
                    deint(t, 0, WL)
                    pad(t)
                ot = ot_pool.tile([P, D * WL], mybir.dt.bfloat16,
                                  name=f"ot{t}", tag="ot")
                if t == 0:
                    # x-pieces (quarter, quarter, half) so compute starts as
                    # soon as the first 160 right elements have landed.
                    for xoff, nx in [(0, 40), (40, 40), (80, 80)]:
                        for s2 in range(2):
                            sub(ot, po, 0, s2, 0, 12, xoff=xoff, nx=nx)
                    out_dma(r0, ot, 0, 24 * WL)
                    out_dma(r0, ot, 24 * WL, 24 * WL)
                elif t < TILES - 2:
                    for s2 in range(2):
                        sub(ot, po, t * WL, s2, 0, 12)
                    out_dma(r0, ot, 0, 24 * WL)
                    out_dma(r0, ot, 24 * WL, 24 * WL)
                elif t < TILES - 1:
                    for qh in range(2):
                        for s2 in range(2):
                            sub(ot, po, t * WL, s2, 6 * qh, 6)
                        out_dma(r0, ot, qh * 24 * WL, 24 * WL)
                else:
                    for q0, nq in TAPER:
                        for s2 in range(2):
                            sub(ot, po, t * WL, s2, q0, nq)
                        out_dma(r0, ot, 4 * q0 * WL, 4 * nq * WL)

    nc.finalize()
    return nc


def _run(left_feature, right_feature, trace=False, **trace_kw):
    global _cached
    left = np.ascontiguousarray(np.asarray(left_feature, dtype=np.float32).reshape(R, WL))
    right = np.ascontiguousarray(np.asarray(right_feature, dtype=np.float32).reshape(R, WR))
    if _cached is None:
        _cached = _build()
    nc = _cached
    in_maps = [
        {"left": left[i * RPC:(i + 1) * RPC], "right": right[i * RPC:(i + 1) * RPC]}
        for i in range(N_CORES)
    ]
    res = run_bass_kernel_spmd(nc, in_maps, list(range(N_CORES)), trace=trace, **trace_kw)
    shards = [np.asarray(res.results[i]["out"]) for i in range(N_CORES)]
    full = np.concatenate(shards, axis=0).reshape(B, C, H, D, WL).transpose(0, 1, 3, 2, 4)
    return np.ascontiguousarray(full, dtype=np.float32), res


def kernel(left_feature, right_feature, max_disp=48, **_ignored):
    assert int(max_disp) == D
    out, _ = _run(left_feature, right_feature, trace=False)
    return out


# revision 7
# speedup vs baseline: 1.0046x; 1.0046x over previous
"""DiffVolumeV2 Trainium2 kernel, v8.

out[b,c,d,h,x] = left[b,c,h,x] - right[b,c,h, clip(4x - d + 1, 0, Wr-1)]
with B=4, C=32, H=80, Wl=160, Wr=640, D=48.  10240 independent (b,c,h)
rows sharded contiguously across 8 cores; 10 tiles of 128 partitions.

Same phase-plane decomposition as v7 (d = 4q+s; plane[r][u] = right[4u+r];
front pad = right[0] realizes the clip), but with the planes stored
slot-permuted (slot s holds phase r_s = s^1) so the s-dependence of every
access is affine over (s2, s1), s = 2*s2 + s1:

    out[(4q+2*s2+s1)*WL + x] = left[x] - plane_slot[2*s2+s1][13 - s2 - q + x]

which lets ONE 3-free-dim TensorTensor instruction per s2 cover 2 phases x
12 q x 160 x (the DVE TT encoding S3S3D3 allows 3 free dims; v7's
scalar_tensor_tensor is S2S2D2 and needed 4 instructions + a dummy scalar).
2 subtract instructions per tile instead of 4; the DVE stream is ~98% dense.

v8 structural changes vs v7:
  * All inputs on the two HWDGE rings (sync + scalar queues): tile 0's
    right in quarters split across both rings + left, then tiles 1-9
    alternating rings.  First subtract starts ~2.5us earlier.
  * All output DMAs issued from the GpSimd SWDGE queue (desc-gen on the
    otherwise idle GpSimd engine), so the Act queue runs nothing but
    deinterleaves/pads and the rings carry inputs only.
  * Tail taper [4,3,2,2,1] q-chunks on the last tile, each chunk's output
    DMA'd on completion.
"""

import numpy as np
from concourse import bacc, bass, tile
from concourse.bass_utils import run_bass_kernel_spmd
from concourse.tile_rust import add_dep_helper
import concourse.mybir as mybir

B, C, H, WL, WR, D = 4, 32, 80, 160, 640, 48
N_CORES = 8
R = B * C * H            # 10240 rows
RPC = R // N_CORES       # 1280 rows per core
P = 128
TILES = RPC // P         # 10
PPAD = 13
PW = PPAD + WL           # 173
PLW = 4 * PW
TAPER = [(0, 4), (4, 3), (7, 2), (9, 1), (10, 1), (11, 1)]

_cached = None


def _build() -> bass.Bass:
    nc = bacc.Bacc()
    left_p = nc.declare_dram_parameter("left", [RPC, WL], mybir.dt.float32, isOutput=False)
    right_p = nc.declare_dram_parameter("right", [RPC, WR], mybir.dt.float32, isOutput=False)
    out_p = nc.declare_dram_parameter("out", [RPC, D, WL], mybir.dt.bfloat16, isOutput=True)
    out_flat = out_p[:].rearrange("r d x -> r (d x)")

    def ap(t, off, dims):
        return bass.AP(t.tensor, t.offset + off, [list(t.ap[0])] + dims)

    chains = {}

    def order(key, inst):
        prev = chains.get(key)
        if prev is not None:
            add_dep_helper(inst.ins, prev.ins, sync=False,
                           reason=f"{key} program order")
        chains[key] = inst
        return inst

    def tensor_tensor(out, in0, in1):
        eng = nc.vector
        return eng.add_instruction(
            mybir.InstTensorTensor(
                name=eng.bass.get_next_instruction_name(),
                op=mybir.AluOpType.subtract,
                ins=[eng.lower_ap(in0), eng.lower_ap(in1)],
                outs=[eng.lower_ap(out)],
            )
        )

    with tile.TileContext(nc) as tc:
        with tc.tile_pool(name="inp", bufs=1) as inp_pool, \
             tc.tile_pool(name="ot", bufs=3) as ot_pool:
            rt_all = inp_pool.tile([P, TILES * WR], mybir.dt.float32)
            lt_all = inp_pool.tile([P, TILES * WL], mybir.dt.float32)
            planes = inp_pool.tile([P, TILES * PLW], mybir.dt.float32)

            ringA, ringB = nc.sync, nc.scalar
            HQ = WR // 4     # 160-element quarter of a right row

            def load_right(eng, t, off, n):
                return eng.dma_start(
                    out=ap(rt_all, t * WR + off, [[1, n]]),
                    in_=bass.AP(right_p[:].tensor, t * P * WR + off,
                                [[WR, P], [1, n]]))

            def load_left(eng, t):
                eng.dma_start(
                    out=ap(lt_all, t * WL, [[1, WL]]),
                    in_=bass.AP(left_p[:].tensor, t * P * WL, [[WL, P], [1, WL]]))

            # Tile 0: right quarters 1,2 + left on ring A (deint of the
            # first quarter gates the first subtract), right half 2 on B.
            load_right(ringA, 0, 0, HQ)
            load_left(ringA, 0)
            load_right(ringA, 0, HQ, HQ)
            rh2 = load_right(ringB, 0, 2 * HQ, 2 * HQ)
            load_right(ringA, 1, 0, WR)
            load_left(ringA, 1)
            load_right(ringB, 2, 0, WR)
            load_left(ringB, 2)

            # Tiles 1-9 via the GpSimd SWDGE queue (desc-gen off the
            # ring-issuing queues, which carry the output stream).
            def load_group(t0, nt):
                first = nc.gpsimd.dma_start(
                    out=ap(rt_all, t0 * WR, [[WR, nt], [1, WR]]),
                    in_=bass.AP(right_p[:].tensor, t0 * P * WR,
                                [[WR, P], [WR * P, nt], [1, WR]]))
                nc.gpsimd.dma_start(
                    out=ap(lt_all, t0 * WL, [[WL, nt], [1, WL]]),
                    in_=bass.AP(left_p[:].tensor, t0 * P * WL,
                                [[WL, P], [WL * P, nt], [1, WL]]))
                return first

            g1 = load_group(3, 3)
            load_group(6, 4)
            add_dep_helper(g1.ins, rh2.ins, sync=True,
                           reason="keep head DMA engines clear of bulk input")

            Copy = mybir.ActivationFunctionType.Copy

            def deint(t, xoff, nx):
                # plane_slot[2*s2+s1][PPAD+xoff+u] = right[4*(xoff+u) + 2*s2+1-s1]
                for s2 in range(2):
                    order("act", nc.scalar.activation(
                        ap(planes, t * PLW + 2 * s2 * PW + PPAD + xoff,
                           [[PW, 2], [1, nx]]),
                        ap(rt_all, t * WR + 4 * xoff + 2 * s2 + 1,
                           [[-1, 2], [4, nx]]),
                        Copy))

            def pad(t):
                order("act", nc.scalar.activation(
                    ap(planes, t * PLW, [[PW, 4], [1, PPAD]]),
                    ap(rt_all, t * WR, [[0, 4], [0, PPAD]]),
                    Copy))

            def sub(ot, po, lt_off, s2, q0, nq, xoff=0, nx=WL):
                return order("dve", tensor_tensor(
                    ap(ot, (4 * q0 + 2 * s2) * WL + xoff,
                       [[WL, 2], [4 * WL, nq], [1, nx]]),
                    ap(lt_all, lt_off + xoff, [[0, 2], [0, nq], [1, nx]]),
                    ap(planes, po + s2 * (2 * PW - 1) + PPAD - q0 + xoff,
                       [[PW, 2], [-1, nq], [1, nx]])))

            ring_i = [0]

            def out_dma(r0, ot, c0, cw):
                eng = ringA if ring_i[0] % 2 == 0 else ringB
                ring_i[0] += 1
                eng.dma_start(out=out_flat[r0:r0 + P, c0:c0 + cw],
                              in_=ot[:, c0:c0 + cw])

            for t in range(TILES):
                r0 = t * P
                po = t * PLW
                if t == 0:
                    pad(0)
                    deint(0, 0, WL // 4)
                    deint(0, WL // 4, WL // 4)
                    deint(0, WL // 2, WL // 2)
                else:
                    deint(t, 0, WL)
                    pad(t)
                ot = ot_pool.tile([P, D * WL], mybir.dt.bfloat16,
                                  name=f"ot{t}", tag="ot")
                if t == 0:
                    # x-pieces (quarter, quarter, half) so compute starts as
                    # soon as the first 160 right elements have landed.
                    for xoff, nx in [(0, 40), (40, 40), (80, 80)]:
                        for s2 in range(2):
                            sub(ot, po, 0, s2, 0, 12, xoff=xoff, nx=nx)
                    out_dma(r0, ot, 0, 24 * WL)
                    out_dma(r0, ot, 24 * WL, 24 * WL)
                elif t < TILES - 2:
                    for s2 in range(2):
                        sub(ot, po, t * WL, s2, 0, 12)
                    out_dma(r0, ot, 0, 24 * WL)
                    out_dma(r0, ot, 24 * WL, 24 * WL)
                elif t < TILES - 1:
                    for qh in range(2):
                        for s2 in range(2):
                            sub(ot, po, t * WL, s2, 6 * qh, 6)
                        out_dma(r0, ot, qh * 24 * WL, 24 * WL)
                else:
                    for q0, nq in TAPER:
                        if (q0, nq) == (11, 1):
                            for s2 in range(2):
                                sub(ot, po, t * WL, s2, q0, nq)
                                out_dma(r0, ot, (44 + 2 * s2) * WL, 2 * WL)
                        else:
                            for s2 in range(2):
                                sub(ot, po, t * WL, s2, q0, nq)
                            out_dma(r0, ot, 4 * q0 * WL, 4 * nq * WL)

    nc.finalize()
    return nc


def _run(left_feature, right_feature, trace=False, **trace_kw):
    global _cached
    left = np.ascontiguousarray(np.asarray(left_feature, dtype=np.float32).reshape(R, WL))
    right = np.ascontiguousarray(np.asarray(right_feature, dtype=np.float32).reshape(R, WR))
    if _cached is None:
        _cached = _build()
    nc = _cached
    in_maps = [
        {"left": left[i * RPC:(i + 1) * RPC], "right": right[i * RPC:(i + 1) * RPC]}
        for i in range(N_CORES)
    ]
    res = run_bass_kernel_spmd(nc, in_maps, list(range(N_CORES)), trace=trace, **trace_kw)
    shards = [np.asarray(res.results[i]["out"]) for i in range(N_CORES)]
    full = np.concatenate(shards, axis=0).reshape(B, C, H, D, WL).transpose(0, 1, 3, 2, 4)
    return np.ascontiguousarray(full, dtype=np.float32), res


def kernel(left_feature, right_feature, max_disp=48, **_ignored):
    assert int(max_disp) == D
    out, _ = _run(left_feature, right_feature, trace=False)
    return out
